# revision 27
# baseline (speedup 1.0000x reference)
"""Trainium2 Bass kernel for nn_Capsule (Efficient-CapsNet style capsule layer).

Math (see reference):
    u[b,k,j,:] = x[b,j,:] @ w[k,j,:,:]            # per-(k,j) 16x16 projection
    t[b,k,:]   = sum_j u[b,k,j,:]
    l[b,k,j]   = <u[b,k,j,:], t[b,k,:]> / sqrt(D)
    c          = softmax_k(l) + bias
    s[b,k,:]   = sum_j c[b,k,j] u[b,k,j,:]
    out        = squash(s)

Sharding: the j (N=2048) contraction axis is split over 8 cores (256 j each),
so each core reads only its w slice once (4.2 MB in bf16).  Cross-core
coupling is a single 64 KB AllReduce of t; the softmax over k is core-local
(all k stay on-core).  The per-core partial s sums are combined on host as
the gather/unshard step, followed by the (tiny) squash.

Per-core device schedule:
  inputs:  host pre-packs bf16 layouts: w as [128=(j4,i),(k,z)] tile-pair
           columns, a block-diagonal x (lets one matmul emit per-j outputs for
           4 j at once), and a dense transposed x for the t matmuls.
  phase 1: 32 accumulating bf16 matmuls -> t_partial[b,(k,z)] over local j
           -> AllReduce(t) -> replicate to t_rep[128,512] (+bf16 cast)
  phase 2: 16 quads (4 tiles of 4 j each; 4-bank PSUM tiles):
           u    = 4x matmul(block-diag-x, w_tile)   # [(4j,b)=128,(k,z)] PSUM
           u_bf = ACT copy-cast PSUM->SBUF bf16
           prod = u_bf * t_rep (DVE bf16 2x); l = reduce_z(prod)  (DVE 1x)
           e = exp(l/4) (ACT); Z = reduce_k(e); c = e/Z          (DVE)
           prod2 = u_bf * bcast_z(c)   (DVE 1x / GpSimd, split)
           s_acc += prod2              (GpSimd, own SBUF port pair)
  tail:    fold quad slots -> s_part[128=(4j,b),512] -> DRAM out
"""

import sys

if "/opt/trn_rl_repo" not in sys.path:
    sys.path.insert(0, "/opt/trn_rl_repo")

import numpy as np
import os

B, N, D_IN = 32, 2048, 16
K, D_OUT = 32, 16
NCORES = 8
NS = N // NCORES          # 256 local j per core
NT = NS // 4              # 64 tiles of 4 j
NT2 = NT // 2             # 32 tile pairs (8 j each) for the t matmuls
KZ = K * D_OUT            # 512
EPS = 1e-20

_CACHE = {}


def _round_fp32r(x):
    """Round-to-nearest fp32 -> fp32r (11 explicit mantissa bits), matching
    the PE's fp32r operand format so pre-rounded data is bit-identical to a
    hardware cast."""
    u = np.ascontiguousarray(x, dtype=np.float32).view(np.uint32)
    r = ((u.astype(np.uint64) + 0x800) & 0xFFFFF000).astype(np.uint32)
    return r.view(np.float32)


def _pack_inputs(x, w, b):
    """Per-core host-side marshaling into the DMA-friendly layouts (bf16)."""
    import ml_dtypes
    bf = ml_dtypes.bfloat16
    xr = x.astype(bf).astype(np.float32)      # [B, N, D_IN]
    wr = w.astype(bf).astype(np.float32)      # [K, N, D_IN, D_OUT]
    per_core = []
    for r in range(NCORES):
        js, je = r * NS, (r + 1) * NS
        # w_host[64h+q, t2*512 + (k*16+z)] = w[k, js+ (2*t2+h)*4 + jr, i, z], q = jr*16+i
        wc = wr[:, js:je]                         # [K, NS, D_IN, D_OUT]
        wc = wc.transpose(1, 2, 0, 3)             # [NS, D_IN, K, D_OUT]  (j, i, k, z)
        wc = wc.reshape(NT, 64, KZ)               # [jt, (jr i), (k z)]
        wc = wc.reshape(NT2, 2, 64, KZ).transpose(1, 2, 0, 3)  # [h, q, t2, c]
        w_host = np.ascontiguousarray(wc.reshape(128, NT2 * KZ)).astype(bf)

        # block-diagonal x for the u matmuls:
        # bdx[64h+q, t2*128 + jr*32 + b] = x[b, j(tile,jr), i] iff q == jr*16+i
        xc = xr[:, js:je, :]                      # [B, NS, D_IN]
        xc = xc.transpose(1, 2, 0)                # [NS, D_IN, B]  (j, i, b)
        bdx = np.zeros((2, 64, NT2, 128), dtype=np.float32)   # [h, q, t2, col]
        xt4 = xc.reshape(NT2, 2, 4, D_IN, B)      # [t2, h, jr, i, b]
        for jr in range(4):
            # rows q = jr*16..jr*16+16, cols jr*32..jr*32+32
            bdx[:, jr * 16:(jr + 1) * 16, :, jr * 32:(jr + 1) * 32] = (
                xt4[:, :, jr].transpose(1, 2, 0, 3)           # [h, i, t2, b]
            )
        bdx_host = np.ascontiguousarray(bdx.reshape(128, NT2 * 128)).astype(bf)

        # dense xT for the t matmuls: xt[jj*16+i, t2*32+b] = x[b, js+t2*8+jj, i]
        xt = xc.reshape(NT2, 8, D_IN, B)          # [t2, jj, i, b]
        xt = xt.transpose(1, 2, 0, 3)             # [jj, i, t2, b]
        xt_host = np.ascontiguousarray(xt.reshape(128, NT2 * B)).astype(bf)

        per_core.append({"w": w_host, "bdx": bdx_host, "xt": xt_host})

    if np.any(b):
        # brep[p=(jr*32+bb), tile*K + k] = b[k, j(tile,jr)]  (replicated over bb)
        for r in range(NCORES):
            js = r * NS
            bc = b[:, js:js + NS, 0]                         # [K, NS]
            br = bc.transpose(1, 0).reshape(NT, 4, 1, K)     # [tile, jr, 1, k]
            br = np.broadcast_to(br, (NT, 4, 32, K))         # replicate over batch
            brep = br.transpose(1, 2, 0, 3).reshape(128, NT * K)
            per_core[r]["brep"] = np.ascontiguousarray(brep, dtype=np.float32)
        with_bias = True
    else:
        with_bias = False
    return per_core, with_bias


def _build(with_bias):
    from concourse import bacc, mybir
    from concourse.tile import TileContext

    f32 = mybir.dt.float32
    bf16d = mybir.dt.bfloat16

    nc = bacc.Bacc("TRN2", target_bir_lowering=False, debug=False,
                   num_devices=NCORES)
    w_in = nc.declare_dram_parameter("w", [128, NT2 * KZ], bf16d, isOutput=False)
    bdx_in = nc.declare_dram_parameter("bdx", [128, NT2 * 128], bf16d, isOutput=False)
    xt_in = nc.declare_dram_parameter("xt", [128, NT2 * B], bf16d, isOutput=False)
    brep_in = None
    if with_bias:
        brep_in = nc.declare_dram_parameter("brep", [128, NT * K], f32, isOutput=False)
    s_out = nc.declare_dram_parameter("s_part", [128, KZ], f32, isOutput=True)

    t_ar_in = nc.dram_tensor("t_ar_in", [32, KZ], f32)
    t_ar_out = nc.dram_tensor("t_ar_out", [32, KZ], f32, addr_space="Shared")

    with TileContext(nc) as tc:
        with (
            tc.tile_pool(name="wp", bufs=1) as wp,
            tc.tile_pool(name="xp", bufs=1) as xp,
            tc.tile_pool(name="sp", bufs=1) as sp,
            tc.tile_pool(name="work", bufs=4) as work,
            tc.tile_pool(name="small", bufs=6) as small,
            tc.tile_pool(name="pu", bufs=2, space="PSUM") as pu,
        ):
            bf16 = mybir.dt.bfloat16
            NQ = NT // 4          # 16 quads of 4 tiles (= 2 tile-pairs each)
            xt_sb = xp.tile([128, NT2 * B], bf16, tag="xt")
            nc.sync.dma_start(out=xt_sb[:, :], in_=xt_in[:, :])
            bdx_sb = xp.tile([128, NT2 * 128], bf16, tag="bdx")
            nc.sync.dma_start(out=bdx_sb[:, :], in_=bdx_in[:, :])
            w_sb = wp.tile([128, NT2 * KZ], bf16, tag="w")
            wq = NT2 * KZ // 2
            for ci in range(2):
                nc.sync.dma_start(out=w_sb[:, ci * wq:(ci + 1) * wq],
                                  in_=w_in[:, ci * wq:(ci + 1) * wq])
            brep_sb = None
            if with_bias:
                brep_sb = xp.tile([128, NT * K], f32, tag="brep")
                nc.sync.dma_start(out=brep_sb[:, :], in_=brep_in[:, :])

            # ---- phase 1: partial t (uses one rotating quad PSUM slot) ----
            t_psum = pu.tile([128, 4 * KZ], f32, tag="u")
            for t2 in range(NT2):
                nc.tensor.matmul(t_psum[0:32, 0:KZ],
                                 xt_sb[:, t2 * B:(t2 + 1) * B],
                                 w_sb[:, t2 * KZ:(t2 + 1) * KZ],
                                 start=(t2 == 0), stop=(t2 == NT2 - 1))
            t_loc = sp.tile([32, KZ], f32, tag="t_loc")
            nc.scalar.copy(t_loc[:, :], t_psum[0:32, 0:KZ])
            nc.sync.dma_start(out=t_ar_in[:, :], in_=t_loc[:, :])
            nc.gpsimd.collective_compute(
                "AllReduce",
                mybir.AluOpType.add,
                replica_groups=[list(range(NCORES))],
                ins=[t_ar_in[:, :].opt()],
                outs=[t_ar_out[:, :].opt()],
            )
            t_rep = sp.tile([128, KZ], f32, tag="t_rep")
            nc.sync.dma_start(out=t_rep[0:32, :], in_=t_ar_out[:, :])
            t_rep_bf = sp.tile([128, KZ], bf16, tag="t_rep_bf")
            nc.scalar.copy(t_rep_bf[0:32, :], t_rep[0:32, :])
            for q in range(1, 4):
                nc.scalar.copy(t_rep_bf[32 * q:32 * q + 32, :],
                               t_rep_bf[0:32, :])

            s_acc_gp = sp.tile([128, 4 * KZ], f32, tag="s_acc_gp")
            nc.gpsimd.memset(s_acc_gp[:, :], 0.0)

            # ---- phase 2: quads of 4 tiles ----
            for q in range(NQ):

                uq = pu.tile([128, 4 * KZ], f32, tag="u")
                for s in range(4):
                    tile = q * 4 + s
                    t2, h = tile // 2, tile % 2
                    nc.tensor.matmul(uq[:, s * KZ:(s + 1) * KZ],
                                     bdx_sb[64 * h:64 * h + 64,
                                            t2 * 128:(t2 + 1) * 128],
                                     w_sb[64 * h:64 * h + 64,
                                          t2 * KZ:(t2 + 1) * KZ],
                                     start=True, stop=True)
                u_bf = work.tile([128, 4 * KZ], bf16, tag="u_bf")
                nc.scalar.copy(u_bf[:, :], uq[:, :])

                # l logits: prod = u * t (bf16, t broadcast over the 4 tiles)
                prod = work.tile([128, 4 * KZ], bf16, tag="prod")
                t_bc = t_rep_bf[:, :].unsqueeze(1).to_broadcast((128, 4, KZ))
                nc.vector.tensor_tensor(
                    prod[:, :].rearrange("p (s c) -> p s c", s=4),
                    u_bf[:, :].rearrange("p (s c) -> p s c", s=4),
                    t_bc, op=mybir.AluOpType.mult)
                # pair-sum z 16->8->4 first (bf16 2x TT) so the 1x-only
                # tensor_reduce sees a quarter of the elements
                ph = work.tile([128, 2 * KZ], bf16, tag="ph")
                pv = prod[:, :].rearrange("p (m h z) -> p m h z", h=2, z=8)
                nc.vector.tensor_tensor(
                    ph[:, :].rearrange("p (m z) -> p m z", z=8),
                    pv[:, :, 0, :], pv[:, :, 1, :], op=mybir.AluOpType.add)
                p4 = work.tile([128, KZ], bf16, tag="p4")
                phv = ph[:, :].rearrange("p (m h z) -> p m h z", h=2, z=4)
                nc.vector.tensor_tensor(
                    p4[:, :].rearrange("p (m z) -> p m z", z=4),
                    phv[:, :, 0, :], phv[:, :, 1, :], op=mybir.AluOpType.add)
                lg = small.tile([128, 4 * K], f32, tag="lg")
                nc.vector.tensor_reduce(
                    lg[:, :],
                    p4[:, :].rearrange("p (m z) -> p m z", z=4),
                    axis=mybir.AxisListType.X, op=mybir.AluOpType.add)
                e = small.tile([128, 4 * K], f32, tag="e")
                nc.scalar.activation(e[:, :], lg[:, :],
                                     mybir.ActivationFunctionType.Exp,
                                     scale=0.25)
                zq = small.tile([128, 4], f32, tag="zq")
                nc.vector.tensor_reduce(
                    zq[:, :],
                    e[:, :].rearrange("p (s k) -> p s k", k=K),
                    axis=mybir.AxisListType.X, op=mybir.AluOpType.add)
                rz = small.tile([128, 4], f32, tag="rz")
                nc.vector.reciprocal(rz[:, :], zq[:, :])
                c = small.tile([128, 4 * K], bf16, tag="c")
                rz_bc = rz[:, :].unsqueeze(-1).to_broadcast((128, 4, K))
                nc.vector.tensor_tensor(
                    c[:, :].rearrange("p (s k) -> p s k", k=K),
                    e[:, :].rearrange("p (s k) -> p s k", k=K),
                    rz_bc, op=mybir.AluOpType.mult)
                if with_bias:
                    nc.vector.tensor_tensor(
                        c[:, :], c[:, :],
                        brep_sb[:, q * 4 * K:(q + 1) * 4 * K],
                        op=mybir.AluOpType.add)

                # s accumulation: ScalarE (otherwise idle here) materializes
                # c replicated over z so the multiply keeps unit strides and
                # runs at DVE 2x; the fp32 running-sum add stays on GpSimd
                # (its own SBUF port pair - 1x/2x_1P DVE ops never contend).
                c_rep = work.tile([128, 4 * KZ], bf16, tag="c_rep")
                c_bc = c[:, :].unsqueeze(-1).to_broadcast((128, 4 * K, D_OUT))
                nc.scalar.copy(c_rep[:, :].rearrange("p (m z) -> p m z", z=D_OUT),
                               c_bc)
                prod2 = work.tile([128, 4 * KZ], bf16, tag="prod2")
                nc.vector.tensor_tensor(prod2[:, :], u_bf[:, :], c_rep[:, :],
                                        op=mybir.AluOpType.mult)
                nc.gpsimd.tensor_tensor(s_acc_gp[:, :], s_acc_gp[:, :],
                                        prod2[:, :], op=mybir.AluOpType.add)

            # ---- tail: fold quad slots; host folds j rows ----
            s_fin = sp.tile([128, KZ], f32, tag="s_fin")
            nc.vector.tensor_reduce(
                s_fin[:, :],
                s_acc_gp[:, :].rearrange("p (s c) -> p c s", s=4),
                axis=mybir.AxisListType.X, op=mybir.AluOpType.add)
            nc.sync.dma_start(out=s_out[:, :], in_=s_fin[:, :])

    nc.compile()
    return nc


def _get_nc(with_bias):
    key = ("nc", with_bias)
    if key not in _CACHE:
        _CACHE[key] = _build(with_bias)
    return _CACHE[key]


def _get_runner(with_bias):
    """Build (once) a cached shard_map-jitted executable for the 8-core SPMD
    kernel, mirroring bass2jax.run_bass_via_pjrt but reusable across calls."""
    key = ("runner", with_bias)
    if key in _CACHE:
        return _CACHE[key]

    import jax
    from jax.sharding import Mesh, PartitionSpec
    from jax.experimental.shard_map import shard_map
    from concourse import mybir
    from concourse import bass2jax
    from concourse.bass2jax import (_bass_exec_p, install_neuronx_cc_hook,
                                    partition_id_tensor)

    install_neuronx_cc_hook()
    nc = _get_nc(with_bias)

    partition_name = nc.partition_id_tensor.name if nc.partition_id_tensor else None
    in_names, out_names, out_avals, zero_shapes = [], [], [], []
    for alloc in nc.m.functions[0].allocations:
        if not isinstance(alloc, mybir.MemoryLocationSet):
            continue
        name = alloc.memorylocations[0].name
        if alloc.kind == "ExternalInput":
            if name != partition_name:
                in_names.append(name)
        elif alloc.kind == "ExternalOutput":
            out_names.append(name)
            shape = tuple(alloc.tensor_shape)
            dtype = mybir.dt.np(alloc.dtype)
            out_avals.append(jax.core.ShapedArray(shape, dtype))
            zero_shapes.append((shape, dtype))
    n_params = len(in_names)
    n_outs = len(out_avals)
    all_in_names = list(in_names) + list(out_names)
    if partition_name is not None:
        all_in_names.append(partition_name)

    def _body(*args):
        operands = list(args)
        if partition_name is not None:
            operands.append(partition_id_tensor())
        outs = _bass_exec_p.bind(
            *operands,
            out_avals=tuple(out_avals),
            in_names=tuple(all_in_names),
            out_names=tuple(out_names),
            lowering_input_output_aliases=(),
            sim_require_finite=True,
            sim_require_nnan=True,
            nc=nc,
        )
        return tuple(outs)

    devices = jax.devices()[:NCORES]
    mesh = Mesh(np.asarray(devices), ("core",))
    in_specs = (PartitionSpec("core"),) * (n_params + n_outs)
    out_specs = (PartitionSpec("core"),) * n_outs
    donate = tuple(range(n_params, n_params + n_outs))
    sharded = jax.jit(
        shard_map(_body, mesh=mesh, in_specs=in_specs, out_specs=out_specs,
                  check_rep=False),
        donate_argnums=donate, keep_unused=True)

    def run(per_core):
        concat_in = [
            np.concatenate([np.asarray(per_core[c][nm]) for c in range(NCORES)], axis=0)
            for nm in in_names
        ]
        concat_zeros = [np.zeros((NCORES * sh[0], *sh[1:]), dt)
                        for sh, dt in zero_shapes]
        out_arrs = sharded(*concat_in, *concat_zeros)
        return [
            {nm: np.asarray(out_arrs[i]).reshape(NCORES, *out_avals[i].shape)[c]
             for i, nm in enumerate(out_names)}
            for c in range(NCORES)
        ]

    _CACHE[key] = run
    return run


def kernel(x, w, b, _run_kwargs=None):
    x = np.asarray(x, dtype=np.float32)
    w = np.asarray(w, dtype=np.float32)
    b = np.asarray(b, dtype=np.float32)

    per_core, with_bias = _pack_inputs(x, w, b)
    results = _get_runner(with_bias)(per_core)

    s = np.zeros((32, KZ), dtype=np.float64)
    for r in range(NCORES):
        s += results[r]["s_part"].astype(np.float64).reshape(4, 32, KZ).sum(0)
    s = s.astype(np.float32).reshape(B, K, D_OUT)

    # efficient squash (host-side finalization of the gathered partials)
    n = np.linalg.norm(s.astype(np.float64), axis=-1, keepdims=True)
    out = (1.0 - 1.0 / (np.exp(n) + EPS)) * (s / (n + EPS))
    return out.astype(np.float32)


# revision 28
# speedup vs baseline: 1.0119x; 1.0119x over previous
"""Trainium2 Bass kernel for nn_Capsule (Efficient-CapsNet style capsule layer).

Math (see reference):
    u[b,k,j,:] = x[b,j,:] @ w[k,j,:,:]            # per-(k,j) 16x16 projection
    t[b,k,:]   = sum_j u[b,k,j,:]
    l[b,k,j]   = <u[b,k,j,:], t[b,k,:]> / sqrt(D)
    c          = softmax_k(l) + bias
    s[b,k,:]   = sum_j c[b,k,j] u[b,k,j,:]
    out        = squash(s)

Sharding: the j (N=2048) contraction axis is split over 8 cores (256 j each),
so each core reads only its w slice once (4.2 MB in bf16).  Cross-core
coupling is a single 64 KB AllReduce of t; the softmax over k is core-local
(all k stay on-core).  The per-core partial s sums are combined on host as
the gather/unshard step, followed by the (tiny) squash.

Per-core device schedule:
  inputs:  host pre-packs bf16 layouts: w as [128=(j4,i),(k,z)] tile-pair
           columns, a block-diagonal x (lets one matmul emit per-j outputs for
           4 j at once), and a dense transposed x for the t matmuls.
  phase 1: 32 accumulating bf16 matmuls -> t_partial[b,(k,z)] over local j
           -> AllReduce(t) -> replicate to t_rep[128,512] (+bf16 cast)
  phase 2: 16 quads (4 tiles of 4 j each; 4-bank PSUM tiles):
           u    = 4x matmul(block-diag-x, w_tile)   # [(4j,b)=128,(k,z)] PSUM
           u_bf = ACT copy-cast PSUM->SBUF bf16
           prod = u_bf * t_rep (DVE bf16 2x); l = reduce_z(prod)  (DVE 1x)
           e = exp(l/4) (ACT); Z = reduce_k(e); c = e/Z          (DVE)
           prod2 = u_bf * bcast_z(c)   (DVE 1x / GpSimd, split)
           s_acc += prod2              (GpSimd, own SBUF port pair)
  tail:    fold quad slots -> s_part[128=(4j,b),512] -> DRAM out
"""

import sys

if "/opt/trn_rl_repo" not in sys.path:
    sys.path.insert(0, "/opt/trn_rl_repo")

import numpy as np
import os

B, N, D_IN = 32, 2048, 16
K, D_OUT = 32, 16
NCORES = 8
NS = N // NCORES          # 256 local j per core
NT = NS // 4              # 64 tiles of 4 j
NT2 = NT // 2             # 32 tile pairs (8 j each) for the t matmuls
KZ = K * D_OUT            # 512
EPS = 1e-20

_CACHE = {}


def _round_fp32r(x):
    """Round-to-nearest fp32 -> fp32r (11 explicit mantissa bits), matching
    the PE's fp32r operand format so pre-rounded data is bit-identical to a
    hardware cast."""
    u = np.ascontiguousarray(x, dtype=np.float32).view(np.uint32)
    r = ((u.astype(np.uint64) + 0x800) & 0xFFFFF000).astype(np.uint32)
    return r.view(np.float32)


def _pack_inputs(x, w, b):
    """Per-core host-side marshaling into the DMA-friendly layouts (bf16)."""
    import ml_dtypes
    bf = ml_dtypes.bfloat16
    xr = x.astype(bf).astype(np.float32)      # [B, N, D_IN]
    wr = w.astype(bf).astype(np.float32)      # [K, N, D_IN, D_OUT]
    per_core = []
    for r in range(NCORES):
        js, je = r * NS, (r + 1) * NS
        # w_host[64h+q, t2*512 + (k*16+z)] = w[k, js+ (2*t2+h)*4 + jr, i, z], q = jr*16+i
        wc = wr[:, js:je]                         # [K, NS, D_IN, D_OUT]
        wc = wc.transpose(1, 2, 0, 3)             # [NS, D_IN, K, D_OUT]  (j, i, k, z)
        wc = wc.reshape(NT, 64, KZ)               # [jt, (jr i), (k z)]
        wc = wc.reshape(NT2, 2, 64, KZ).transpose(1, 2, 0, 3)  # [h, q, t2, c]
        w_host = np.ascontiguousarray(wc.reshape(128, NT2 * KZ)).astype(bf)

        # block-diagonal x for the u matmuls:
        # bdx[64h+q, t2*128 + jr*32 + b] = x[b, j(tile,jr), i] iff q == jr*16+i
        xc = xr[:, js:je, :]                      # [B, NS, D_IN]
        xc = xc.transpose(1, 2, 0)                # [NS, D_IN, B]  (j, i, b)
        bdx = np.zeros((2, 64, NT2, 128), dtype=np.float32)   # [h, q, t2, col]
        xt4 = xc.reshape(NT2, 2, 4, D_IN, B)      # [t2, h, jr, i, b]
        for jr in range(4):
            # rows q = jr*16..jr*16+16, cols jr*32..jr*32+32
            bdx[:, jr * 16:(jr + 1) * 16, :, jr * 32:(jr + 1) * 32] = (
                xt4[:, :, jr].transpose(1, 2, 0, 3)           # [h, i, t2, b]
            )
        bdx_host = np.ascontiguousarray(bdx.reshape(128, NT2 * 128)).astype(bf)

        # dense xT for the t matmuls: xt[jj*16+i, t2*32+b] = x[b, js+t2*8+jj, i]
        xt = xc.reshape(NT2, 8, D_IN, B)          # [t2, jj, i, b]
        xt = xt.transpose(1, 2, 0, 3)             # [jj, i, t2, b]
        xt_host = np.ascontiguousarray(xt.reshape(128, NT2 * B)).astype(bf)

        per_core.append({"w": w_host, "bdx": bdx_host, "xt": xt_host})

    if np.any(b):
        # brep[p=(jr*32+bb), tile*K + k] = b[k, j(tile,jr)]  (replicated over bb)
        for r in range(NCORES):
            js = r * NS
            bc = b[:, js:js + NS, 0]                         # [K, NS]
            br = bc.transpose(1, 0).reshape(NT, 4, 1, K)     # [tile, jr, 1, k]
            br = np.broadcast_to(br, (NT, 4, 32, K))         # replicate over batch
            brep = br.transpose(1, 2, 0, 3).reshape(128, NT * K)
            per_core[r]["brep"] = np.ascontiguousarray(brep, dtype=np.float32)
        with_bias = True
    else:
        with_bias = False
    return per_core, with_bias


def _build(with_bias):
    from concourse import bacc, mybir
    from concourse.tile import TileContext

    f32 = mybir.dt.float32
    bf16d = mybir.dt.bfloat16

    nc = bacc.Bacc("TRN2", target_bir_lowering=False, debug=False,
                   num_devices=NCORES)
    w_in = nc.declare_dram_parameter("w", [128, NT2 * KZ], bf16d, isOutput=False)
    bdx_in = nc.declare_dram_parameter("bdx", [128, NT2 * 128], bf16d, isOutput=False)
    xt_in = nc.declare_dram_parameter("xt", [128, NT2 * B], bf16d, isOutput=False)
    brep_in = None
    if with_bias:
        brep_in = nc.declare_dram_parameter("brep", [128, NT * K], f32, isOutput=False)
    s_out = nc.declare_dram_parameter("s_part", [128, 4 * KZ], f32, isOutput=True)

    t_ar_in = nc.dram_tensor("t_ar_in", [32, KZ], f32)
    t_ar_out = nc.dram_tensor("t_ar_out", [32, KZ], f32, addr_space="Shared")

    with TileContext(nc) as tc:
        with (
            tc.tile_pool(name="wp", bufs=1) as wp,
            tc.tile_pool(name="xp", bufs=1) as xp,
            tc.tile_pool(name="sp", bufs=1) as sp,
            tc.tile_pool(name="work", bufs=5) as work,
            tc.tile_pool(name="small", bufs=6) as small,
            tc.tile_pool(name="pu", bufs=2, space="PSUM") as pu,
        ):
            bf16 = mybir.dt.bfloat16
            NQ = NT // 4          # 16 quads of 4 tiles (= 2 tile-pairs each)
            xt_sb = xp.tile([128, NT2 * B], bf16, tag="xt")
            nc.sync.dma_start(out=xt_sb[:, :], in_=xt_in[:, :])
            bdx_sb = xp.tile([128, NT2 * 128], bf16, tag="bdx")
            nc.sync.dma_start(out=bdx_sb[:, :], in_=bdx_in[:, :])
            w_sb = wp.tile([128, NT2 * KZ], bf16, tag="w")
            wq = NT2 * KZ // 2
            for ci in range(2):
                nc.sync.dma_start(out=w_sb[:, ci * wq:(ci + 1) * wq],
                                  in_=w_in[:, ci * wq:(ci + 1) * wq])
            brep_sb = None
            if with_bias:
                brep_sb = xp.tile([128, NT * K], f32, tag="brep")
                nc.sync.dma_start(out=brep_sb[:, :], in_=brep_in[:, :])

            # ---- phase 1: partial t (uses one rotating quad PSUM slot) ----
            t_psum = pu.tile([128, 4 * KZ], f32, tag="u")
            for t2 in range(NT2):
                nc.tensor.matmul(t_psum[0:32, 0:KZ],
                                 xt_sb[:, t2 * B:(t2 + 1) * B],
                                 w_sb[:, t2 * KZ:(t2 + 1) * KZ],
                                 start=(t2 == 0), stop=(t2 == NT2 - 1))
            t_loc = sp.tile([32, KZ], f32, tag="t_loc")
            nc.scalar.copy(t_loc[:, :], t_psum[0:32, 0:KZ])
            nc.sync.dma_start(out=t_ar_in[:, :], in_=t_loc[:, :])
            nc.gpsimd.collective_compute(
                "AllReduce",
                mybir.AluOpType.add,
                replica_groups=[list(range(NCORES))],
                ins=[t_ar_in[:, :].opt()],
                outs=[t_ar_out[:, :].opt()],
            )
            t_rep = sp.tile([128, KZ], f32, tag="t_rep")
            nc.sync.dma_start(out=t_rep[0:32, :], in_=t_ar_out[:, :])
            t_rep_bf = sp.tile([128, KZ], bf16, tag="t_rep_bf")
            nc.scalar.copy(t_rep_bf[0:32, :], t_rep[0:32, :])
            for q in range(1, 4):
                nc.scalar.copy(t_rep_bf[32 * q:32 * q + 32, :],
                               t_rep_bf[0:32, :])

            s_acc_gp = sp.tile([128, 4 * KZ], f32, tag="s_acc_gp")
            nc.gpsimd.memset(s_acc_gp[:, :], 0.0)

            # ---- phase 2: quads of 4 tiles ----
            for q in range(NQ):

                uq = pu.tile([128, 4 * KZ], f32, tag="u")
                for s in range(4):
                    tile = q * 4 + s
                    t2, h = tile // 2, tile % 2
                    nc.tensor.matmul(uq[:, s * KZ:(s + 1) * KZ],
                                     bdx_sb[64 * h:64 * h + 64,
                                            t2 * 128:(t2 + 1) * 128],
                                     w_sb[64 * h:64 * h + 64,
                                          t2 * KZ:(t2 + 1) * KZ],
                                     start=True, stop=True)
                u_bf = work.tile([128, 4 * KZ], bf16, tag="u_bf")
                nc.scalar.copy(u_bf[:, :], uq[:, :])

                # l logits: prod = u * t (bf16, t broadcast over the 4 tiles)
                prod = work.tile([128, 4 * KZ], bf16, tag="prod")
                t_bc = t_rep_bf[:, :].unsqueeze(1).to_broadcast((128, 4, KZ))
                nc.vector.tensor_tensor(
                    prod[:, :].rearrange("p (s c) -> p s c", s=4),
                    u_bf[:, :].rearrange("p (s c) -> p s c", s=4),
                    t_bc, op=mybir.AluOpType.mult)
                # pair-sum z 16->8->4 first (bf16 2x TT) so the 1x-only
                # tensor_reduce sees a quarter of the elements
                ph = work.tile([128, 2 * KZ], bf16, tag="ph")
                pv = prod[:, :].rearrange("p (m h z) -> p m h z", h=2, z=8)
                nc.vector.tensor_tensor(
                    ph[:, :].rearrange("p (m z) -> p m z", z=8),
                    pv[:, :, 0, :], pv[:, :, 1, :], op=mybir.AluOpType.add)
                p4 = work.tile([128, KZ], bf16, tag="p4")
                phv = ph[:, :].rearrange("p (m h z) -> p m h z", h=2, z=4)
                nc.vector.tensor_tensor(
                    p4[:, :].rearrange("p (m z) -> p m z", z=4),
                    phv[:, :, 0, :], phv[:, :, 1, :], op=mybir.AluOpType.add)
                lg = small.tile([128, 4 * K], f32, tag="lg")
                nc.vector.tensor_reduce(
                    lg[:, :],
                    p4[:, :].rearrange("p (m z) -> p m z", z=4),
                    axis=mybir.AxisListType.X, op=mybir.AluOpType.add)
                e = small.tile([128, 4 * K], f32, tag="e")
                nc.scalar.activation(e[:, :], lg[:, :],
                                     mybir.ActivationFunctionType.Exp,
                                     scale=0.25)
                zq = small.tile([128, 4], f32, tag="zq")
                nc.vector.tensor_reduce(
                    zq[:, :],
                    e[:, :].rearrange("p (s k) -> p s k", k=K),
                    axis=mybir.AxisListType.X, op=mybir.AluOpType.add)
                rz = small.tile([128, 4], f32, tag="rz")
                nc.vector.reciprocal(rz[:, :], zq[:, :])
                c = small.tile([128, 4 * K], bf16, tag="c")
                rz_bc = rz[:, :].unsqueeze(-1).to_broadcast((128, 4, K))
                nc.vector.tensor_tensor(
                    c[:, :].rearrange("p (s k) -> p s k", k=K),
                    e[:, :].rearrange("p (s k) -> p s k", k=K),
                    rz_bc, op=mybir.AluOpType.mult)
                if with_bias:
                    nc.vector.tensor_tensor(
                        c[:, :], c[:, :],
                        brep_sb[:, q * 4 * K:(q + 1) * 4 * K],
                        op=mybir.AluOpType.add)

                # s accumulation: ScalarE (otherwise idle here) materializes
                # c replicated over z so the multiply keeps unit strides and
                # runs at DVE 2x; the fp32 running-sum add stays on GpSimd
                # (its own SBUF port pair - 1x/2x_1P DVE ops never contend).
                c_rep = work.tile([128, 4 * KZ], bf16, tag="c_rep")
                c_bc = c[:, :].unsqueeze(-1).to_broadcast((128, 4 * K, D_OUT))
                nc.scalar.copy(c_rep[:, :].rearrange("p (m z) -> p m z", z=D_OUT),
                               c_bc)
                prod2 = work.tile([128, 4 * KZ], bf16, tag="prod2")
                nc.vector.tensor_tensor(prod2[:, :], u_bf[:, :], c_rep[:, :],
                                        op=mybir.AluOpType.mult)
                nc.gpsimd.tensor_tensor(s_acc_gp[:, :], s_acc_gp[:, :],
                                        prod2[:, :], op=mybir.AluOpType.add)

            # ---- tail: ship the accumulator unfolded; host folds the quad
            # slots and j rows together with the cross-core sum ----
            nc.sync.dma_start(out=s_out[:, :], in_=s_acc_gp[:, :])

    nc.compile()
    return nc


def _get_nc(with_bias):
    key = ("nc", with_bias)
    if key not in _CACHE:
        _CACHE[key] = _build(with_bias)
    return _CACHE[key]


def _get_runner(with_bias):
    """Build (once) a cached shard_map-jitted executable for the 8-core SPMD
    kernel, mirroring bass2jax.run_bass_via_pjrt but reusable across calls."""
    key = ("runner", with_bias)
    if key in _CACHE:
        return _CACHE[key]

    import jax
    from jax.sharding import Mesh, PartitionSpec
    from jax.experimental.shard_map import shard_map
    from concourse import mybir
    from concourse import bass2jax
    from concourse.bass2jax import (_bass_exec_p, install_neuronx_cc_hook,
                                    partition_id_tensor)

    install_neuronx_cc_hook()
    nc = _get_nc(with_bias)

    partition_name = nc.partition_id_tensor.name if nc.partition_id_tensor else None
    in_names, out_names, out_avals, zero_shapes = [], [], [], []
    for alloc in nc.m.functions[0].allocations:
        if not isinstance(alloc, mybir.MemoryLocationSet):
            continue
        name = alloc.memorylocations[0].name
        if alloc.kind == "ExternalInput":
            if name != partition_name:
                in_names.append(name)
        elif alloc.kind == "ExternalOutput":
            out_names.append(name)
            shape = tuple(alloc.tensor_shape)
            dtype = mybir.dt.np(alloc.dtype)
            out_avals.append(jax.core.ShapedArray(shape, dtype))
            zero_shapes.append((shape, dtype))
    n_params = len(in_names)
    n_outs = len(out_avals)
    all_in_names = list(in_names) + list(out_names)
    if partition_name is not None:
        all_in_names.append(partition_name)

    def _body(*args):
        operands = list(args)
        if partition_name is not None:
            operands.append(partition_id_tensor())
        outs = _bass_exec_p.bind(
            *operands,
            out_avals=tuple(out_avals),
            in_names=tuple(all_in_names),
            out_names=tuple(out_names),
            lowering_input_output_aliases=(),
            sim_require_finite=True,
            sim_require_nnan=True,
            nc=nc,
        )
        return tuple(outs)

    devices = jax.devices()[:NCORES]
    mesh = Mesh(np.asarray(devices), ("core",))
    in_specs = (PartitionSpec("core"),) * (n_params + n_outs)
    out_specs = (PartitionSpec("core"),) * n_outs
    donate = tuple(range(n_params, n_params + n_outs))
    sharded = jax.jit(
        shard_map(_body, mesh=mesh, in_specs=in_specs, out_specs=out_specs,
                  check_rep=False),
        donate_argnums=donate, keep_unused=True)

    def run(per_core):
        concat_in = [
            np.concatenate([np.asarray(per_core[c][nm]) for c in range(NCORES)], axis=0)
            for nm in in_names
        ]
        concat_zeros = [np.zeros((NCORES * sh[0], *sh[1:]), dt)
                        for sh, dt in zero_shapes]
        out_arrs = sharded(*concat_in, *concat_zeros)
        return [
            {nm: np.asarray(out_arrs[i]).reshape(NCORES, *out_avals[i].shape)[c]
             for i, nm in enumerate(out_names)}
            for c in range(NCORES)
        ]

    _CACHE[key] = run
    return run


def kernel(x, w, b, _run_kwargs=None):
    x = np.asarray(x, dtype=np.float32)
    w = np.asarray(w, dtype=np.float32)
    b = np.asarray(b, dtype=np.float32)

    per_core, with_bias = _pack_inputs(x, w, b)
    results = _get_runner(with_bias)(per_core)

    s = np.zeros((32, KZ), dtype=np.float64)
    for r in range(NCORES):
        sp_r = results[r]["s_part"].astype(np.float64)
        # [128=(4jr,32b), 4slots*KZ]: fold quad slots then jr partition groups
        s += sp_r.reshape(4, 32, 4, KZ).sum(axis=(0, 2))
    s = s.astype(np.float32).reshape(B, K, D_OUT)

    # efficient squash (host-side finalization of the gathered partials)
    n = np.linalg.norm(s.astype(np.float64), axis=-1, keepdims=True)
    out = (1.0 - 1.0 / (np.exp(n) + EPS)) * (s / (n + EPS))
    return out.astype(np.float32)
